# revision 44
# baseline (speedup 1.0000x reference)
"""Trainium2 Bass kernel for AdaptiveScaledDotProductAttention.

Sharding: DP=4 over batch x TP=2 over heads (8 NeuronCores).
Core c handles batch c//2, head-group g=c%2 (heads 8g..8g+7).
Each core projects q/k/v/s for its 8 heads over the full sequence,
runs attention, and computes a PARTIAL output projection against the
full Wo (its 8 heads' rows). The host sums the two partials per batch
during the unshard/gather step -- no on-chip collective at all.

Layout strategy:
 - Host passes inputs pre-transposed (feature-major x^T) and pre-cast
   to bf16.
 - q/k/s projected feature-major with HEAD PAIRS stacked across the
   128 partitions -> QK runs as two concurrent 64-row-tiled matmuls
   (tile_position auto-derived from base partition 0 / 64).
 - AV uses E (exp scores) as the STATIONARY operand and V (+ ones
   column for the softmax denominator) as moving, token-major output.
 - s_tok (token-major s, for the adaptive-language term) is derived
   from sT2 via SBUF->SBUF DMA XBAR transposes -- no PE/DVE cost.
 - att_tok is written bf16 by the epilogue and transposed to
   feature-major via DMA XBAR transposes (replaces 32 PE transposes
   + PSUM evacuations). For each token block the pair-0..2 chunks are
   transposed as soon as those pairs' epilogues finish; only the tiny
   pair-3 chunk rides in the critical tail.
 - Softmax epilogue computed as att = (av + el*s) * rc with three
   WIDE broadcast tensor_tensor ops per unit (stride-0 rc/el views)
   instead of 16 narrow per-(head, block) ops.
 - The per-query language logit rides into column 65 of the same PSUM
   accumulator via a tiny K=64 matmul over p = q*s; the denominator
   rides in column 64 via an all-ones column of the moving V operand.
 - Unit-level software pipeline: qk_part(u+1)'s score matmuls + exps
   are emitted before av_part(u) so the scalar engine always has a
   full unit of exp work queued while the PE runs AV + fillers.
 - Input DMAs alternate between the two HWDGE queues (sync + act) so
   descriptor issue overlaps transfer.
"""

import numpy as np

H, DK, DV, DM = 16, 64, 64, 1024
B, N = 4, 1024
SCALE = float(1.0 / np.sqrt(DK))
NCORES = 8
HLOC = 8          # heads per core
NPAIR = HLOC // 2  # head pairs per core
FLOC = HLOC * DK  # 512 local features

_CACHE = {}


def _build(with_biases):
    import concourse.bass as bass
    import concourse.tile as tile
    from concourse import bacc, mybir

    f32 = mybir.dt.float32
    bf16 = mybir.dt.bfloat16
    Exp = mybir.ActivationFunctionType.Exp
    mult = mybir.AluOpType.mult
    add = mybir.AluOpType.add

    nc = bacc.Bacc("TRN2", target_bir_lowering=False, debug=False,
                   num_devices=NCORES)

    def din(name, shape, dt=bf16):
        return nc.dram_tensor(name, shape, dt, kind="ExternalInput").ap()

    # feature-major (transposed) activations, bf16, staged on host
    xq = din("xqT", [DM, N])
    xk = din("xkT", [DM, N])
    xv = din("xvT", [DM, N])
    xs = din("xsT", [DM, N])
    wq = din("wq", [DM, FLOC])
    wk = din("wk", [DM, FLOC])
    wv = din("wv", [DM, FLOC])
    ws = din("ws", [DM, FLOC])
    wo = din("wo", [FLOC, DM])      # local heads' rows of Wo, full dm
    if with_biases:
        bqp = din("bqp", [128, NPAIR], f32)   # pair-major per-partition bias
        bkp = din("bkp", [128, NPAIR], f32)
        bsp = din("bsp", [128, NPAIR], f32)
        bvr = din("bvr", [1, FLOC], f32)      # row bias for token-major v
    out = nc.dram_tensor("out", [N, DM], bf16, kind="ExternalOutput").ap()

    from contextlib import ExitStack
    with ExitStack() as top:
        tc = top.enter_context(tile.TileContext(nc))

        persist = top.enter_context(tc.tile_pool(name="persist", bufs=1))
        # inputs (feature-major planes) + weights
        xq_sb = persist.tile([128, 8, N], bf16)
        xk_sb = persist.tile([128, 8, N], bf16)
        xv_sb = persist.tile([128, 8, N], bf16)
        xs_sb = persist.tile([128, 8, N], bf16)
        wq_sb = persist.tile([128, 8, FLOC], bf16)
        wk_sb = persist.tile([128, 8, FLOC], bf16)
        wv_sb = persist.tile([128, 8, FLOC], bf16)
        ws_sb = persist.tile([128, 8, FLOC], bf16)
        wo_sb = persist.tile([128, 4, DM], bf16)
        # projections: head-pair-stacked feature-major, token-major v/s
        qT2 = persist.tile([128, NPAIR, N], bf16)
        kT2 = persist.tile([128, NPAIR, N], bf16)
        sT2 = persist.tile([128, NPAIR, N], bf16)
        s_tok = persist.tile([128, 8, FLOC], bf16)
        vaug = persist.tile([128, 8, HLOC, DV + 1], bf16)
        att_tok = persist.tile([128, 8, FLOC], bf16)
        att_feat = persist.tile([128, 4, N], bf16)
        ones = persist.tile([128, 1], bf16)

        nc.vector.memset(ones[:, :], 1.0)
        nc.vector.memset(vaug[:, :, :, DV:DV + 1], 1.0)

        if with_biases:
            bq_sb = persist.tile([128, NPAIR], f32)
            bk_sb = persist.tile([128, NPAIR], f32)
            bs_sb = persist.tile([128, NPAIR], f32)
            nc.sync.dma_start(out=bq_sb, in_=bqp)
            nc.sync.dma_start(out=bk_sb, in_=bkp)
            nc.sync.dma_start(out=bs_sb, in_=bsp)
            bvrow = persist.tile([1, FLOC], f32)
            nc.sync.dma_start(out=bvrow, in_=bvr)
            ones_row = persist.tile([1, 128], bf16)
            nc.vector.memset(ones_row[:, :], 1.0)

        # Stream inputs in consumption order. The K-side loads issue in
        # parallel from the ACT hwdge queue: the startup ramp is DMA
        # *issue*-bound (~0.7us serialized per instruction) and ACT is
        # idle until the first exp (~16us). Everything later stays on
        # sync so ACT time is never stolen from the exp stream.
        def dma_in(dst, src_ap, eng=None):
            (eng or nc.sync).dma_start(out=dst, in_=src_ap)

        def _pc(dst, src, j0, j1, c0=0, c1=None, eng=None):
            ap = src.rearrange("(j p) c -> p j c", p=128)
            c1 = ap.shape[2] if c1 is None else c1
            dma_in(dst[:, j0:j1, c0:c1], ap[:, j0:j1, c0:c1], eng)

        # pair-0 weight column blocks first so the very first projection
        # matmuls are gated only on the x half-planes
        _pc(wq_sb, wq, 0, 8, 0, 128)
        _pc(wk_sb, wk, 0, 8, 0, 128, eng=nc.scalar)
        for j0 in range(0, 8, 2):
            _pc(xk_sb, xk, j0, j0 + 2, eng=nc.scalar)
        for j0 in range(0, 8, 2):
            _pc(xq_sb, xq, j0, j0 + 2, 0, 512)
        for j0 in range(0, 8, 2):
            _pc(xq_sb, xq, j0, j0 + 2, 512, 1024)
        for j0 in range(0, 8, 4):
            _pc(xv_sb, xv, j0, j0 + 4)
            _pc(wv_sb, wv, j0, j0 + 4)
        for j0 in range(0, 8, 4):
            _pc(xs_sb, xs, j0, j0 + 4)
            _pc(ws_sb, ws, j0, j0 + 4)
        _pc(wq_sb, wq, 0, 8, 128, 512)
        _pc(wk_sb, wk, 0, 8, 128, 512)
        for j0 in range(0, 4, 2):
            _pc(wo_sb, wo, j0, j0 + 2)

        ppool = top.enter_context(tc.tile_pool(name="ppool", bufs=3))
        epool = top.enter_context(tc.tile_pool(name="epool", bufs=8))
        smpool = top.enter_context(tc.tile_pool(name="smpool", bufs=3))
        stpool = top.enter_context(tc.tile_pool(name="stpool", bufs=3))
        ostpool = top.enter_context(tc.tile_pool(name="ostpool", bufs=6))
        ps_o = top.enter_context(
            tc.tile_pool(name="ps_o", bufs=2, space="PSUM"))
        ps_sc = top.enter_context(
            tc.tile_pool(name="ps_sc", bufs=1, space="PSUM"))
        ps_av = top.enter_context(
            tc.tile_pool(name="ps_av", bufs=1, space="PSUM"))

        if with_biases:
            # token-major v-bias plane built once via K=1 matmul broadcast
            bvb = persist.tile([1, FLOC], bf16)
            nc.vector.tensor_copy(out=bvb, in_=bvrow)
            ps = ps_o.tile([128, FLOC], f32, tag="ps_proj")
            nc.tensor.matmul(ps, ones_row, bvb, start=True, stop=True)
            bv_plane = persist.tile([128, FLOC], f32)
            nc.vector.tensor_copy(out=bv_plane, in_=ps)

        # feature-major projection of one head pair half (q/k/s)
        def proj_half(x_sb, w_sb, t, dst, bias, half):
            ps = ps_o.tile([128, 512], f32, tag="ps_proj")
            for j in range(8):
                nc.tensor.matmul(
                    ps, w_sb[:, j, t * 128:(t + 1) * 128],
                    x_sb[:, j, half * 512:(half + 1) * 512],
                    start=(j == 0), stop=(j == 7))
            dsl = dst[:, t, half * 512:(half + 1) * 512]
            nc.vector.tensor_copy(out=dsl, in_=ps)
            if with_biases:
                nc.vector.tensor_scalar_add(dsl, dsl, bias[:, t:t + 1])

        # token-major projection of v, one token block
        def proj_v(tb):
            ps = ps_o.tile([128, 512], f32, tag="ps_proj")
            for j in range(8):
                nc.tensor.matmul(
                    ps, xv_sb[:, j, tb * 128:(tb + 1) * 128],
                    wv_sb[:, j, :], start=(j == 0), stop=(j == 7))
            if with_biases:
                t2 = stpool.tile([128, FLOC], f32, tag="bias_tmp")
                nc.vector.tensor_add(t2, ps, bv_plane)
                nc.vector.tensor_copy(
                    out=vaug[:, tb, :, 0:DV],
                    in_=t2.rearrange("p (h d) -> p h d", h=HLOC))
            else:
                nc.vector.tensor_copy(
                    out=vaug[:, tb, :, 0:DV],
                    in_=ps.rearrange("p (h d) -> p h d", h=HLOC))

        p_map = {}

        def mk_p(t, qc):
            def f():
                p = ppool.tile([128, 512], bf16, tag="p")
                qs = slice(qc * 512, (qc + 1) * 512)
                nc.vector.tensor_mul(p, qT2[:, t, qs], sT2[:, t, qs])
                p_map[(t, qc)] = p
            return f

        # s_tok[:, qc*4+di, t*128+f] = sT2[f, t, qc*512+di*128+q]
        def dmaT_s(t, qc):
            def f():
                nc.sync.dma_start_transpose(
                    out=s_tok[:, qc * 4:(qc + 1) * 4,
                              t * 128:(t + 1) * 128],
                    in_=sT2[:, t, qc * 512:(qc + 1) * 512])
            return f

        # transpose att_tok chunks of pairs [t0, t1) for token block tb
        def dmaT_att(tb, t0, t1, eng=None):
            (eng or nc.sync).dma_start_transpose(
                out=att_feat[:, t0:t1, tb * 128:(tb + 1) * 128],
                in_=att_tok[:, tb, t0 * 128:t1 * 128])

        # partial out-projection of one token block (att_feat ready)
        def out_proj(tb):
            for half in range(2):
                po = ps_o.tile([128, 512], f32, tag="ps_proj")
                for fc in range(4):
                    nc.tensor.matmul(
                        po,
                        att_feat[:, fc, tb * 128:(tb + 1) * 128],
                        wo_sb[:, fc, half * 512:(half + 1) * 512],
                        start=(fc == 0), stop=(fc == 3))
                ost = ostpool.tile([128, 512], bf16, tag="ostage")
                nc.vector.tensor_copy(out=ost, in_=po)
                nc.sync.dma_start(
                    out=out[tb * 128:(tb + 1) * 128,
                            half * 512:(half + 1) * 512],
                    in_=ost)

        def phase_c(tb):
            # pair 0..2 chunks were transposed right after their pairs'
            # epilogues; only the pair-3 chunk is transposed here
            dmaT_att(tb, 3, 4)
            out_proj(tb)

        def dmaT3(tb):
            return lambda: dmaT_att(tb, 3, 4)

        OPf = lambda tb: (lambda: out_proj(tb))

        bq = bq_sb if with_biases else None
        bk = bk_sb if with_biases else None
        bs = bs_sb if with_biases else None

        Qf = lambda t, h: (lambda: proj_half(xq_sb, wq_sb, t, qT2, bq, h))
        Kf = lambda t, h: (lambda: proj_half(xk_sb, wk_sb, t, kT2, bk, h))
        Sf = lambda t, h: (lambda: proj_half(xs_sb, ws_sb, t, sT2, bs, h))
        Vf = lambda tb: (lambda: proj_v(tb))
        PCf = lambda tb: (lambda: phase_c(tb))

        es_map = {}

        def qk_part(t, qc, fillers):
            # 2048-wide exp chunks: one sc tile + one ACTIVATE per pair
            # of key blocks, halving the per-instruction ACT overhead
            qs = slice(qc * 512, (qc + 1) * 512)
            Es = []
            nf = 0
            for g in range(4):
                sc = ps_sc.tile([128, 4, 512], f32, tag="sc")
                for j in range(2):
                    kb = 2 * g + j
                    for h2 in range(2):
                        hp = slice(h2 * 64, (h2 + 1) * 64)
                        nc.tensor.matmul(
                            sc[:, 2 * j + h2, :],
                            kT2[hp, t, kb * 128:(kb + 1) * 128],
                            qT2[hp, t, qs],
                            start=True, stop=True)
                E = epool.tile([128, 4, 512], bf16, tag="E")
                nc.scalar.activation(
                    E.rearrange("p a b -> p (a b)"),
                    sc.rearrange("p a b -> p (a b)"),
                    Exp, scale=SCALE)
                Es.append(E)
                if nf < len(fillers):
                    fillers[nf]()
                    nf += 1
            while nf < len(fillers):
                fillers[nf]()
                nf += 1
            es_map[(t, qc)] = Es

        def av_part(t, qc):
            Es = es_map.pop((t, qc))
            p = p_map.pop((t, qc))
            av = ps_av.tile([128, 2, 4, DV + 2], f32, tag="av")
            # language logits ride as column 65 of each q-block
            for h2 in range(2):
                hp = slice(h2 * 64, (h2 + 1) * 64)
                for qb in range(4):
                    nc.tensor.matmul(
                        av[:, h2, qb, DV + 1:DV + 2],
                        p[hp, qb * 128:(qb + 1) * 128],
                        ones[hp, :],
                        start=True, stop=True)
            # el = exp(lang) runs on ACT while the AV matmuls stream
            el = smpool.tile([128, 2, 4], f32, tag="el")
            nc.scalar.activation(el, av[:, :, :, DV + 1], Exp, scale=SCALE)
            # tmp = s_tok * el (broadcast) -- DVE, overlaps AV
            tbs = slice(qc * 4, (qc + 1) * 4)
            sv = s_tok[:, tbs, t * 128:(t + 1) * 128].rearrange(
                "p b (a d) -> p a b d", a=2)
            el_b = el[:, :, :].unsqueeze(3).broadcast_to([128, 2, 4, DV])
            tmp = stpool.tile([128, 2, 4, DV], f32, tag="tmp")
            nc.vector.tensor_tensor(out=tmp, in0=sv, in1=el_b, op=mult)

            # AV: E stationary (full 128x128), vaug+ones moving.
            # NOTE: each (h2, qb) accumulation group must run to
            # completion before the next group's START -- interleaving
            # groups (kb-outer) corrupts the accumulation on HW.
            for qb in range(4):
                for h2 in range(2):
                    for kb in range(8):
                        nc.tensor.matmul(
                            av[:, h2, qb, 0:DV + 1],
                            Es[kb // 2][:, 2 * (kb % 2) + h2,
                                        qb * 128:(qb + 1) * 128],
                            vaug[:, kb, 2 * t + h2, :],
                            start=(kb == 0), stop=(kb == 7))

            # softmax epilogue: att = (av + el*s) * rc, token-major
            den = smpool.tile([128, 2, 4], f32, tag="den")
            nc.vector.tensor_tensor(
                out=den, in0=av[:, :, :, DV], in1=el, op=add)
            rc = smpool.tile([128, 2, 4], f32, tag="rc")
            nc.vector.reciprocal(rc, den)
            nc.vector.tensor_tensor(
                out=tmp, in0=av[:, :, :, 0:DV], in1=tmp, op=add)
            rc_b = rc[:, :, :].unsqueeze(3).broadcast_to([128, 2, 4, DV])
            attv = att_tok[:, tbs, t * 128:(t + 1) * 128].rearrange(
                "p b (a d) -> p a b d", a=2)
            nc.vector.tensor_tensor(
                out=attv, in0=tmp, in1=rc_b, op=mult)

        def run(fs):
            for f in fs:
                f()

        # ---- software-pipelined emission schedule ----
        # prologue: q half 0 + k (both halves) of pair 0
        Qf(0, 0)()
        Kf(0, 0)()
        Kf(0, 1)()
        qk_part(0, 0, [Qf(0, 1), Vf(0)])
        qk_part(0, 1, [Vf(1), Vf(2), Vf(3), Vf(4),
                       Sf(0, 0), mk_p(0, 0), dmaT_s(0, 0)])
        run([Vf(5), Vf(6), Vf(7)])
        av_part(0, 0)
        run([Qf(1, 0), Kf(1, 0), Kf(1, 1)])
        qk_part(1, 0, [Sf(0, 1), mk_p(0, 1), dmaT_s(0, 1)])
        av_part(0, 1)
        run([Qf(1, 1), Sf(1, 0)])
        qk_part(1, 1, [mk_p(1, 0), dmaT_s(1, 0), Qf(2, 0), Kf(2, 0)])
        av_part(1, 0)
        run([Kf(2, 1), Sf(1, 1)])
        qk_part(2, 0, [mk_p(1, 1), dmaT_s(1, 1), Qf(2, 1), Sf(2, 0)])
        av_part(1, 1)
        run([mk_p(2, 0), dmaT_s(2, 0), Qf(3, 0)])
        qk_part(2, 1, [Kf(3, 0), Kf(3, 1), Sf(2, 1)])
        av_part(2, 0)
        # pair 0..2 chunks of qc=0 token blocks -> transpose early
        for tb in range(0, 4):
            dmaT_att(tb, 0, 3)
        run([mk_p(2, 1), dmaT_s(2, 1), Qf(3, 1)])
        qk_part(3, 0, [Sf(3, 0), mk_p(3, 0), dmaT_s(3, 0)])
        av_part(2, 1)
        for tb in range(4, 8):
            dmaT_att(tb, 0, 3)
        qk_part(3, 1, [Sf(3, 1), mk_p(3, 1), dmaT_s(3, 1)])
        av_part(3, 0)
        # batch the qc=0 pair-3 transposes, then dense out-proj blocks
        run([dmaT3(0), dmaT3(1), dmaT3(2), dmaT3(3)])
        run([OPf(0), OPf(1), OPf(2)])
        av_part(3, 1)
        # tail transposes go on the ACT hwdge queue (idle: exps done)
        # so they don't head-of-line-block the out DMAs on sync
        for tb in range(4, 8):
            dmaT_att(tb, 3, 4, eng=nc.scalar)
        run([OPf(3), OPf(4), OPf(5), OPf(6), OPf(7)])

    nc.compile()
    return nc


def _get_nc(with_biases):
    key = ("nc", with_biases)
    if key not in _CACHE:
        _CACHE[key] = _build(with_biases)
    return _CACHE[key]


def kernel(queries, keys, values, language_signals,
           Wq, b_q, Wk, b_k, Wv, b_v, Ws, b_s, Wo, b_o):
    from concourse.bass_utils import run_bass_kernel_spmd
    import ml_dtypes

    bf = ml_dtypes.bfloat16
    with_biases = any(
        np.any(np.asarray(b)) for b in (b_q, b_k, b_v, b_s, b_o))
    nc = _get_nc(with_biases)

    def bias_pairs(b, hs):
        # [512] feature bias -> [128, 4] pair-major per-partition layout
        return np.ascontiguousarray(
            np.asarray(b[hs], np.float32).reshape(4, 128).T)

    in_maps = []
    for core in range(NCORES):
        b, g = core // 2, core % 2
        hs = slice(FLOC * g, FLOC * (g + 1))
        im = {
            "xqT": np.ascontiguousarray(np.asarray(queries[b]).T, dtype=bf),
            "xkT": np.ascontiguousarray(np.asarray(keys[b]).T, dtype=bf),
            "xvT": np.ascontiguousarray(np.asarray(values[b]).T, dtype=bf),
            "xsT": np.ascontiguousarray(
                np.asarray(language_signals[b]).T, dtype=bf),
            "wq": np.ascontiguousarray(Wq[:, hs], dtype=bf),
            "wk": np.ascontiguousarray(Wk[:, hs], dtype=bf),
            "wv": np.ascontiguousarray(Wv[:, hs], dtype=bf),
            "ws": np.ascontiguousarray(Ws[:, hs], dtype=bf),
            "wo": np.ascontiguousarray(Wo[hs, :], dtype=bf),
        }
        if with_biases:
            im.update({
                "bqp": bias_pairs(b_q, hs),
                "bkp": bias_pairs(b_k, hs),
                "bsp": bias_pairs(b_s, hs),
                "bvr": np.ascontiguousarray(
                    np.asarray(b_v[hs], np.float32).reshape(1, -1)),
            })
        in_maps.append(im)
    _CACHE["last_in_maps"] = in_maps
    res = run_bass_kernel_spmd(nc, in_maps, list(range(NCORES))).results
    full = np.empty((B, N, DM), np.float32)
    for b in range(B):
        full[b] = (np.asarray(res[2 * b]["out"], np.float32)
                   + np.asarray(res[2 * b + 1]["out"], np.float32))
    full += np.asarray(b_o, np.float32)
    return full


# revision 45
# speedup vs baseline: 1.0101x; 1.0101x over previous
"""Trainium2 Bass kernel for AdaptiveScaledDotProductAttention.

Sharding: DP=4 over batch x TP=2 over heads (8 NeuronCores).
Core c handles batch c//2, head-group g=c%2 (heads 8g..8g+7).
Each core projects q/k/v/s for its 8 heads over the full sequence,
runs attention, and computes a PARTIAL output projection against the
full Wo (its 8 heads' rows). The host sums the two partials per batch
during the unshard/gather step -- no on-chip collective at all.

Layout strategy:
 - Host passes inputs pre-transposed (feature-major x^T) and pre-cast
   to bf16.
 - q/k/s projected feature-major with HEAD PAIRS stacked across the
   128 partitions -> QK runs as two concurrent 64-row-tiled matmuls
   (tile_position auto-derived from base partition 0 / 64).
 - AV uses E (exp scores) as the STATIONARY operand and V (+ ones
   column for the softmax denominator) as moving, token-major output.
 - s_tok (token-major s, for the adaptive-language term) is derived
   from sT2 via SBUF->SBUF DMA XBAR transposes -- no PE/DVE cost.
 - att_tok is written bf16 by the epilogue and transposed to
   feature-major via DMA XBAR transposes (replaces 32 PE transposes
   + PSUM evacuations). For each token block the pair-0..2 chunks are
   transposed as soon as those pairs' epilogues finish; only the tiny
   pair-3 chunk rides in the critical tail.
 - Softmax epilogue computed as att = (av + el*s) * rc with three
   WIDE broadcast tensor_tensor ops per unit (stride-0 rc/el views)
   instead of 16 narrow per-(head, block) ops.
 - The per-query language logit rides into column 65 of the same PSUM
   accumulator via a tiny K=64 matmul over p = q*s; the denominator
   rides in column 64 via an all-ones column of the moving V operand.
 - Unit-level software pipeline: qk_part(u+1)'s score matmuls + exps
   are emitted before av_part(u) so the scalar engine always has a
   full unit of exp work queued while the PE runs AV + fillers.
 - Input DMAs alternate between the two HWDGE queues (sync + act) so
   descriptor issue overlaps transfer.
"""

import numpy as np

H, DK, DV, DM = 16, 64, 64, 1024
B, N = 4, 1024
SCALE = float(1.0 / np.sqrt(DK))
NCORES = 8
HLOC = 8          # heads per core
NPAIR = HLOC // 2  # head pairs per core
FLOC = HLOC * DK  # 512 local features

_CACHE = {}


def _build(with_biases):
    import concourse.bass as bass
    import concourse.tile as tile
    from concourse import bacc, mybir

    f32 = mybir.dt.float32
    bf16 = mybir.dt.bfloat16
    Exp = mybir.ActivationFunctionType.Exp
    mult = mybir.AluOpType.mult
    add = mybir.AluOpType.add

    nc = bacc.Bacc("TRN2", target_bir_lowering=False, debug=False,
                   num_devices=NCORES)

    def din(name, shape, dt=bf16):
        return nc.dram_tensor(name, shape, dt, kind="ExternalInput").ap()

    # feature-major (transposed) activations, bf16, staged on host
    xq = din("xqT", [DM, N])
    xk = din("xkT", [DM, N])
    xv = din("xvT", [DM, N])
    xs = din("xsT", [DM, N])
    wq = din("wq", [DM, FLOC])
    wk = din("wk", [DM, FLOC])
    wv = din("wv", [DM, FLOC])
    ws = din("ws", [DM, FLOC])
    wo = din("wo", [FLOC, DM])      # local heads' rows of Wo, full dm
    if with_biases:
        bqp = din("bqp", [128, NPAIR], f32)   # pair-major per-partition bias
        bkp = din("bkp", [128, NPAIR], f32)
        bsp = din("bsp", [128, NPAIR], f32)
        bvr = din("bvr", [1, FLOC], f32)      # row bias for token-major v
    out = nc.dram_tensor("out", [N, DM], bf16, kind="ExternalOutput").ap()

    from contextlib import ExitStack
    with ExitStack() as top:
        tc = top.enter_context(tile.TileContext(nc))

        persist = top.enter_context(tc.tile_pool(name="persist", bufs=1))
        # inputs (feature-major planes) + weights
        xq_sb = persist.tile([128, 8, N], bf16)
        xk_sb = persist.tile([128, 8, N], bf16)
        xv_sb = persist.tile([128, 8, N], bf16)
        xs_sb = persist.tile([128, 8, N], bf16)
        wq_sb = persist.tile([128, 8, FLOC], bf16)
        wk_sb = persist.tile([128, 8, FLOC], bf16)
        wv_sb = persist.tile([128, 8, FLOC], bf16)
        ws_sb = persist.tile([128, 8, FLOC], bf16)
        wo_sb = persist.tile([128, 4, DM], bf16)
        # projections: head-pair-stacked feature-major, token-major v/s
        qT2 = persist.tile([128, NPAIR, N], bf16)
        kT2 = persist.tile([128, NPAIR, N], bf16)
        sT2 = persist.tile([128, NPAIR, N], bf16)
        s_tok = persist.tile([128, 8, FLOC], bf16)
        vaug = persist.tile([128, 8, HLOC, DV + 1], bf16)
        att_tok = persist.tile([128, 8, FLOC], bf16)
        att_feat = persist.tile([128, 4, N], bf16)
        ones = persist.tile([128, 1], bf16)

        nc.vector.memset(ones[:, :], 1.0)
        nc.vector.memset(vaug[:, :, :, DV:DV + 1], 1.0)

        if with_biases:
            bq_sb = persist.tile([128, NPAIR], f32)
            bk_sb = persist.tile([128, NPAIR], f32)
            bs_sb = persist.tile([128, NPAIR], f32)
            nc.sync.dma_start(out=bq_sb, in_=bqp)
            nc.sync.dma_start(out=bk_sb, in_=bkp)
            nc.sync.dma_start(out=bs_sb, in_=bsp)
            bvrow = persist.tile([1, FLOC], f32)
            nc.sync.dma_start(out=bvrow, in_=bvr)
            ones_row = persist.tile([1, 128], bf16)
            nc.vector.memset(ones_row[:, :], 1.0)

        # Stream inputs in consumption order. The K-side loads issue in
        # parallel from the ACT hwdge queue: the startup ramp is DMA
        # *issue*-bound (~0.7us serialized per instruction) and ACT is
        # idle until the first exp (~16us). Everything later stays on
        # sync so ACT time is never stolen from the exp stream.
        def dma_in(dst, src_ap, eng=None):
            (eng or nc.sync).dma_start(out=dst, in_=src_ap)

        def _pc(dst, src, j0, j1, c0=0, c1=None, eng=None):
            ap = src.rearrange("(j p) c -> p j c", p=128)
            c1 = ap.shape[2] if c1 is None else c1
            dma_in(dst[:, j0:j1, c0:c1], ap[:, j0:j1, c0:c1], eng)

        # pair-0 weight column blocks first so the very first projection
        # matmuls are gated only on the x half-planes
        _pc(wq_sb, wq, 0, 8, 0, 128)
        _pc(wk_sb, wk, 0, 8, 0, 128, eng=nc.scalar)
        for j0 in range(0, 8, 2):
            _pc(xk_sb, xk, j0, j0 + 2, eng=nc.scalar)
        for j0 in range(0, 8, 2):
            _pc(xq_sb, xq, j0, j0 + 2, 0, 512)
        for j0 in range(0, 8, 2):
            _pc(xq_sb, xq, j0, j0 + 2, 512, 1024)
        for j0 in range(0, 8, 4):
            _pc(xv_sb, xv, j0, j0 + 4)
            _pc(wv_sb, wv, j0, j0 + 4)
        for j0 in range(0, 8, 4):
            _pc(xs_sb, xs, j0, j0 + 4)
            _pc(ws_sb, ws, j0, j0 + 4)
        _pc(wq_sb, wq, 0, 8, 128, 512)
        _pc(wk_sb, wk, 0, 8, 128, 512)
        for j0 in range(0, 4, 2):
            _pc(wo_sb, wo, j0, j0 + 2)

        ppool = top.enter_context(tc.tile_pool(name="ppool", bufs=3))
        epool = top.enter_context(tc.tile_pool(name="epool", bufs=8))
        smpool = top.enter_context(tc.tile_pool(name="smpool", bufs=3))
        stpool = top.enter_context(tc.tile_pool(name="stpool", bufs=3))
        ostpool = top.enter_context(tc.tile_pool(name="ostpool", bufs=6))
        ps_o = top.enter_context(
            tc.tile_pool(name="ps_o", bufs=2, space="PSUM"))
        ps_sc = top.enter_context(
            tc.tile_pool(name="ps_sc", bufs=1, space="PSUM"))
        ps_av = top.enter_context(
            tc.tile_pool(name="ps_av", bufs=1, space="PSUM"))

        if with_biases:
            # token-major v-bias plane built once via K=1 matmul broadcast
            bvb = persist.tile([1, FLOC], bf16)
            nc.vector.tensor_copy(out=bvb, in_=bvrow)
            ps = ps_o.tile([128, FLOC], f32, tag="ps_proj")
            nc.tensor.matmul(ps, ones_row, bvb, start=True, stop=True)
            bv_plane = persist.tile([128, FLOC], f32)
            nc.vector.tensor_copy(out=bv_plane, in_=ps)

        # feature-major projection of one head pair half (q/k/s)
        def proj_half(x_sb, w_sb, t, dst, bias, half):
            ps = ps_o.tile([128, 512], f32, tag="ps_proj")
            for j in range(8):
                nc.tensor.matmul(
                    ps, w_sb[:, j, t * 128:(t + 1) * 128],
                    x_sb[:, j, half * 512:(half + 1) * 512],
                    start=(j == 0), stop=(j == 7))
            dsl = dst[:, t, half * 512:(half + 1) * 512]
            nc.vector.tensor_copy(out=dsl, in_=ps)
            if with_biases:
                nc.vector.tensor_scalar_add(dsl, dsl, bias[:, t:t + 1])

        # token-major projection of v, one token block
        def proj_v(tb):
            ps = ps_o.tile([128, 512], f32, tag="ps_proj")
            for j in range(8):
                nc.tensor.matmul(
                    ps, xv_sb[:, j, tb * 128:(tb + 1) * 128],
                    wv_sb[:, j, :], start=(j == 0), stop=(j == 7))
            if with_biases:
                t2 = stpool.tile([128, FLOC], f32, tag="bias_tmp")
                nc.vector.tensor_add(t2, ps, bv_plane)
                nc.vector.tensor_copy(
                    out=vaug[:, tb, :, 0:DV],
                    in_=t2.rearrange("p (h d) -> p h d", h=HLOC))
            else:
                nc.vector.tensor_copy(
                    out=vaug[:, tb, :, 0:DV],
                    in_=ps.rearrange("p (h d) -> p h d", h=HLOC))

        p_map = {}

        def mk_p(t, qc):
            def f():
                p = ppool.tile([128, 512], bf16, tag="p")
                qs = slice(qc * 512, (qc + 1) * 512)
                nc.vector.tensor_mul(p, qT2[:, t, qs], sT2[:, t, qs])
                p_map[(t, qc)] = p
            return f

        # s_tok[:, qc*4+di, t*128+f] = sT2[f, t, qc*512+di*128+q]
        def dmaT_s(t, qc):
            def f():
                nc.sync.dma_start_transpose(
                    out=s_tok[:, qc * 4:(qc + 1) * 4,
                              t * 128:(t + 1) * 128],
                    in_=sT2[:, t, qc * 512:(qc + 1) * 512])
            return f

        # transpose att_tok chunks of pairs [t0, t1) for token block tb
        def dmaT_att(tb, t0, t1, eng=None):
            (eng or nc.sync).dma_start_transpose(
                out=att_feat[:, t0:t1, tb * 128:(tb + 1) * 128],
                in_=att_tok[:, tb, t0 * 128:t1 * 128])

        # partial out-projection of one token block (att_feat ready)
        def out_proj(tb):
            for half in range(2):
                po = ps_o.tile([128, 512], f32, tag="ps_proj")
                for fc in range(4):
                    nc.tensor.matmul(
                        po,
                        att_feat[:, fc, tb * 128:(tb + 1) * 128],
                        wo_sb[:, fc, half * 512:(half + 1) * 512],
                        start=(fc == 0), stop=(fc == 3))
                ost = ostpool.tile([128, 512], bf16, tag="ostage")
                nc.vector.tensor_copy(out=ost, in_=po)
                nc.sync.dma_start(
                    out=out[tb * 128:(tb + 1) * 128,
                            half * 512:(half + 1) * 512],
                    in_=ost)

        def phase_c(tb):
            # pair 0..2 chunks were transposed right after their pairs'
            # epilogues; only the pair-3 chunk is transposed here
            dmaT_att(tb, 3, 4)
            out_proj(tb)

        def dmaT3(tb):
            return lambda: dmaT_att(tb, 3, 4)

        OPf = lambda tb: (lambda: out_proj(tb))

        bq = bq_sb if with_biases else None
        bk = bk_sb if with_biases else None
        bs = bs_sb if with_biases else None

        Qf = lambda t, h: (lambda: proj_half(xq_sb, wq_sb, t, qT2, bq, h))
        Kf = lambda t, h: (lambda: proj_half(xk_sb, wk_sb, t, kT2, bk, h))
        Sf = lambda t, h: (lambda: proj_half(xs_sb, ws_sb, t, sT2, bs, h))
        Vf = lambda tb: (lambda: proj_v(tb))
        PCf = lambda tb: (lambda: phase_c(tb))

        es_map = {}

        def qk_part(t, qc, fillers):
            # 2048-wide exp chunks: one sc tile + one ACTIVATE per pair
            # of key blocks, halving the per-instruction ACT overhead
            qs = slice(qc * 512, (qc + 1) * 512)
            Es = []
            nf = 0
            for g in range(4):
                sc = ps_sc.tile([128, 4, 512], f32, tag="sc")
                for j in range(2):
                    kb = 2 * g + j
                    for h2 in range(2):
                        hp = slice(h2 * 64, (h2 + 1) * 64)
                        nc.tensor.matmul(
                            sc[:, 2 * j + h2, :],
                            kT2[hp, t, kb * 128:(kb + 1) * 128],
                            qT2[hp, t, qs],
                            start=True, stop=True)
                E = epool.tile([128, 4, 512], bf16, tag="E")
                nc.scalar.activation(
                    E.rearrange("p a b -> p (a b)"),
                    sc.rearrange("p a b -> p (a b)"),
                    Exp, scale=SCALE)
                Es.append(E)
                if nf < len(fillers):
                    fillers[nf]()
                    nf += 1
            while nf < len(fillers):
                fillers[nf]()
                nf += 1
            es_map[(t, qc)] = Es

        def av_part(t, qc):
            Es = es_map.pop((t, qc))
            p = p_map.pop((t, qc))
            av = ps_av.tile([128, 2, 4, DV + 2], f32, tag="av")
            # language logits ride as column 65 of each q-block
            for h2 in range(2):
                hp = slice(h2 * 64, (h2 + 1) * 64)
                for qb in range(4):
                    nc.tensor.matmul(
                        av[:, h2, qb, DV + 1:DV + 2],
                        p[hp, qb * 128:(qb + 1) * 128],
                        ones[hp, :],
                        start=True, stop=True)
            # el = exp(lang) runs on ACT while the AV matmuls stream
            el = smpool.tile([128, 2, 4], f32, tag="el")
            nc.scalar.activation(el, av[:, :, :, DV + 1], Exp, scale=SCALE)
            # tmp = s_tok * el (broadcast) -- DVE, overlaps AV
            tbs = slice(qc * 4, (qc + 1) * 4)
            sv = s_tok[:, tbs, t * 128:(t + 1) * 128].rearrange(
                "p b (a d) -> p a b d", a=2)
            el_b = el[:, :, :].unsqueeze(3).broadcast_to([128, 2, 4, DV])
            tmp = stpool.tile([128, 2, 4, DV], f32, tag="tmp")
            nc.vector.tensor_tensor(out=tmp, in0=sv, in1=el_b, op=mult)

            # AV: E stationary (full 128x128), vaug+ones moving.
            # NOTE: each (h2, qb) accumulation group must run to
            # completion before the next group's START -- interleaving
            # groups (kb-outer) corrupts the accumulation on HW.
            # Softmax epilogue att = (av + el*s) * rc is split by
            # qb-halves: each half's den/rc/att only depends on its own
            # accumulation groups, so the first half's epilogue (DVE)
            # overlaps the second half's AV matmuls (PE).
            den = smpool.tile([128, 2, 4], f32, tag="den")
            rc = smpool.tile([128, 2, 4], f32, tag="rc")
            rc_b = rc[:, :, :].unsqueeze(3).broadcast_to([128, 2, 4, DV])
            attv = att_tok[:, tbs, t * 128:(t + 1) * 128].rearrange(
                "p b (a d) -> p a b d", a=2)
            for hb in range(2):
                for qb in (2 * hb, 2 * hb + 1):
                    for h2 in range(2):
                        for kb in range(8):
                            nc.tensor.matmul(
                                av[:, h2, qb, 0:DV + 1],
                                Es[kb // 2][:, 2 * (kb % 2) + h2,
                                            qb * 128:(qb + 1) * 128],
                                vaug[:, kb, 2 * t + h2, :],
                                start=(kb == 0), stop=(kb == 7))
                qbs = slice(2 * hb, 2 * hb + 2)
                nc.vector.tensor_tensor(
                    out=den[:, :, qbs], in0=av[:, :, qbs, DV],
                    in1=el[:, :, qbs], op=add)
                nc.vector.reciprocal(rc[:, :, qbs], den[:, :, qbs])
                nc.vector.tensor_tensor(
                    out=tmp[:, :, qbs, :], in0=av[:, :, qbs, 0:DV],
                    in1=tmp[:, :, qbs, :], op=add)
                nc.vector.tensor_tensor(
                    out=attv[:, :, qbs, :], in0=tmp[:, :, qbs, :],
                    in1=rc_b[:, :, qbs, :], op=mult)

        def run(fs):
            for f in fs:
                f()

        # ---- software-pipelined emission schedule ----
        # prologue: q half 0 + k (both halves) of pair 0
        Qf(0, 0)()
        Kf(0, 0)()
        Kf(0, 1)()
        qk_part(0, 0, [Qf(0, 1), Vf(0)])
        qk_part(0, 1, [Vf(1), Vf(2), Vf(3), Vf(4),
                       Sf(0, 0), mk_p(0, 0), dmaT_s(0, 0)])
        run([Vf(5), Vf(6), Vf(7)])
        av_part(0, 0)
        run([Qf(1, 0), Kf(1, 0), Kf(1, 1)])
        qk_part(1, 0, [Sf(0, 1), mk_p(0, 1), dmaT_s(0, 1)])
        av_part(0, 1)
        run([Qf(1, 1), Sf(1, 0)])
        qk_part(1, 1, [mk_p(1, 0), dmaT_s(1, 0), Qf(2, 0), Kf(2, 0)])
        av_part(1, 0)
        run([Kf(2, 1), Sf(1, 1)])
        qk_part(2, 0, [mk_p(1, 1), dmaT_s(1, 1), Qf(2, 1), Sf(2, 0)])
        av_part(1, 1)
        run([mk_p(2, 0), dmaT_s(2, 0), Qf(3, 0)])
        qk_part(2, 1, [Kf(3, 0), Kf(3, 1), Sf(2, 1)])
        av_part(2, 0)
        # pair 0..2 chunks of qc=0 token blocks -> transpose early
        for tb in range(0, 4):
            dmaT_att(tb, 0, 3)
        run([mk_p(2, 1), dmaT_s(2, 1), Qf(3, 1)])
        qk_part(3, 0, [Sf(3, 0), mk_p(3, 0), dmaT_s(3, 0)])
        av_part(2, 1)
        for tb in range(4, 8):
            dmaT_att(tb, 0, 3)
        qk_part(3, 1, [Sf(3, 1), mk_p(3, 1), dmaT_s(3, 1)])
        av_part(3, 0)
        # batch the qc=0 pair-3 transposes, then dense out-proj blocks
        run([dmaT3(0), dmaT3(1), dmaT3(2), dmaT3(3)])
        run([OPf(0), OPf(1), OPf(2)])
        av_part(3, 1)
        # tail transposes go on the ACT hwdge queue (idle: exps done)
        # so they don't head-of-line-block the out DMAs on sync
        for tb in range(4, 8):
            dmaT_att(tb, 3, 4, eng=nc.scalar)
        run([OPf(3), OPf(4), OPf(5), OPf(6), OPf(7)])

    nc.compile()
    return nc


def _get_nc(with_biases):
    key = ("nc", with_biases)
    if key not in _CACHE:
        _CACHE[key] = _build(with_biases)
    return _CACHE[key]


def kernel(queries, keys, values, language_signals,
           Wq, b_q, Wk, b_k, Wv, b_v, Ws, b_s, Wo, b_o):
    from concourse.bass_utils import run_bass_kernel_spmd
    import ml_dtypes

    bf = ml_dtypes.bfloat16
    with_biases = any(
        np.any(np.asarray(b)) for b in (b_q, b_k, b_v, b_s, b_o))
    nc = _get_nc(with_biases)

    def bias_pairs(b, hs):
        # [512] feature bias -> [128, 4] pair-major per-partition layout
        return np.ascontiguousarray(
            np.asarray(b[hs], np.float32).reshape(4, 128).T)

    in_maps = []
    for core in range(NCORES):
        b, g = core // 2, core % 2
        hs = slice(FLOC * g, FLOC * (g + 1))
        im = {
            "xqT": np.ascontiguousarray(np.asarray(queries[b]).T, dtype=bf),
            "xkT": np.ascontiguousarray(np.asarray(keys[b]).T, dtype=bf),
            "xvT": np.ascontiguousarray(np.asarray(values[b]).T, dtype=bf),
            "xsT": np.ascontiguousarray(
                np.asarray(language_signals[b]).T, dtype=bf),
            "wq": np.ascontiguousarray(Wq[:, hs], dtype=bf),
            "wk": np.ascontiguousarray(Wk[:, hs], dtype=bf),
            "wv": np.ascontiguousarray(Wv[:, hs], dtype=bf),
            "ws": np.ascontiguousarray(Ws[:, hs], dtype=bf),
            "wo": np.ascontiguousarray(Wo[hs, :], dtype=bf),
        }
        if with_biases:
            im.update({
                "bqp": bias_pairs(b_q, hs),
                "bkp": bias_pairs(b_k, hs),
                "bsp": bias_pairs(b_s, hs),
                "bvr": np.ascontiguousarray(
                    np.asarray(b_v[hs], np.float32).reshape(1, -1)),
            })
        in_maps.append(im)
    _CACHE["last_in_maps"] = in_maps
    res = run_bass_kernel_spmd(nc, in_maps, list(range(NCORES))).results
    full = np.empty((B, N, DM), np.float32)
    for b in range(B):
        full[b] = (np.asarray(res[2 * b]["out"], np.float32)
                   + np.asarray(res[2 * b + 1]["out"], np.float32))
    full += np.asarray(b_o, np.float32)
    return full
